# revision 7
# baseline (speedup 1.0000x reference)
"""EHR ontology GNN kernel for Trainium2 (8 NeuronCores, SPMD).

Strategy: only the first node of each of the 2048 patient graphs feeds the
output cosine, so each core traces the 3-hop dependency cone of its 256
graphs (host-side index work) and computes only that cone. Ontology GATs are
sharded by all_emb row; the assembled 32001x128 embedding table is the single
AllGather. GCN/GAT layers share one device pattern: dma_gather padded edge
slots -> per-slot scale (static norm or on-device softmax) -> strided
segment-sum -> per-128-node tile matmul (transpose/matmul/transpose) ->
indirect scatter-write into the next layer's table. W factors out of the
aggregation (linearity), so all gathers move raw 512B node rows.
"""
import os
import numpy as np

P = 128
HID = 128
CHUNK = 1024
NQ = 4
NEG = np.float32(-1e30)

LAST_EXEC_TIME_NS = None


def _pow2ceil(x):
    return np.maximum(1, 2 ** np.ceil(np.log2(np.maximum(x, 1))).astype(np.int64))


class LayerSpec:
    """Host-side arrays for one gather/aggregate/matmul layer on one core."""

    def __init__(self, is_gat):
        self.is_gat = is_gat
        self.contig = False
        self.classes = []          # [(D, NB)] AFTER uniformization
        self.idx = None            # int16 [S] slot gather rows (stream order)
        self.scale = None          # f32 [S] (GCN) slot scales
        self.mask = None           # f32 [S] (GAT) additive mask
        self.outsc = None          # f32 [ntiles,128] per-tile per-partition scale
        self.rowoff = None         # int32 [ntiles,128] scatter rows
        self.renum = None          # map real dst node -> table row (dict-free arrays)


def _build_layer(dst_nodes, out_rows, e_src, e_dst, src_row_of, self_row_of,
                 slot_scale_src=None, slot_scale_self=None, out_scale=None,
                 is_gat=False):
    """dst_nodes: node ids producing output rows; out_rows: table row of each.
    e_src/e_dst: edges with e_dst in dst_nodes (dst may repeat via rows).
    src_row_of(v)->gather row. Returns per-core LayerSpec (pre-uniformize)."""
    nd = len(dst_nodes)
    # per-dst-row edge lists: e_dst here given as ROW index into dst_nodes
    deg = np.bincount(e_dst, minlength=nd).astype(np.int64)
    degp = deg + 1
    D = _pow2ceil(degp)
    assert D.max(initial=1) <= 32, D.max()
    order = np.argsort(D, kind="stable")
    spec = LayerSpec(is_gat)
    idx_parts, sc_parts, mk_parts, outsc_parts, rowoff_parts = [], [], [], [], []
    classes = []
    # per-dst edge grouping
    eord = np.argsort(e_dst, kind="stable")
    es_sorted = e_src[eord]
    starts = np.zeros(nd + 1, np.int64)
    np.cumsum(np.bincount(e_dst, minlength=nd), out=starts[1:])
    sels = []
    for Dv in np.unique(D[order]):
        sel = order[D[order] == Dv]
        sels.append(sel)
        n = len(sel)
        gb = max(1, 8 // Dv)
        nb = -(-n // P)
        nb = -(-nb // gb) * gb  # NB*D multiple of 8
        npad = nb * P
        slots = np.zeros((npad, Dv), np.int64)
        scm = np.zeros((npad, Dv), np.float32)
        mkm = np.full((npad, Dv), NEG, np.float32)
        # self slot
        slots[:n, 0] = self_row_of[dst_nodes[sel]]
        mkm[:n, 0] = 0.0
        if slot_scale_self is not None:
            scm[:n, 0] = slot_scale_self[dst_nodes[sel]]
        # edge slots
        dcount = deg[sel]
        rep = np.repeat(np.arange(n), dcount)
        pos = np.concatenate([np.arange(c) for c in dcount]) if n else np.zeros(0, np.int64)
        gidx = np.concatenate([np.arange(starts[i], starts[i + 1]) for i in sel]) if n else np.zeros(0, np.int64)
        srcs = es_sorted[gidx]
        slots[rep, 1 + pos] = src_row_of[srcs]
        mkm[rep, 1 + pos] = 0.0
        if slot_scale_src is not None:
            scm[rep, 1 + pos] = slot_scale_src[srcs]
        # stream order: [(block n, slot d, part p)]
        def stream(m):
            return m.reshape(nb, P, Dv).transpose(0, 2, 1).reshape(-1)
        idx_parts.append(stream(slots))
        sc_parts.append(stream(scm))
        mk_parts.append(stream(mkm))
        osc = np.ones((nb, P), np.float32)
        if out_scale is not None:
            o = np.ones(npad, np.float32)
            o[:n] = out_scale[dst_nodes[sel]]
            osc = o.reshape(nb, P)
        ro = np.empty(npad, np.int64)
        ro[:n] = out_rows[sel]
        classes.append((int(Dv), nb, n))
        outsc_parts.append(osc)
        rowoff_parts.append(ro.reshape(nb, P))
    spec.classes = classes
    spec._parts = (idx_parts, sc_parts, mk_parts, outsc_parts, rowoff_parts)
    spec._sels = sels
    spec._nreal = nd
    return spec


def _uniformize(specs, pad_base=None):
    """Pad 8 per-core specs to a common class structure. Returns common
    classes [(D, NB)], ntiles, and fills spec.idx/scale/mask/outsc/rowoff."""
    allD = sorted({c[0] for s in specs for c in s.classes})
    common = []
    for Dv in allD:
        nb = max((c[1] for s in specs for c in s.classes if c[0] == Dv), default=0)
        gb = max(1, 8 // Dv)
        nb = -(-nb // gb) * gb
        common.append((Dv, nb))
    ntiles = sum(nb for _, nb in common)
    for s in specs:
        idx_l, sc_l, mk_l, os_l, ro_l = [], [], [], [], []
        next_pad_row = s._nreal if pad_base is None else pad_base
        for Dv, NB in common:
            have = [i for i, c in enumerate(s.classes) if c[0] == Dv]
            if have:
                i = have[0]
                idx, sc, mk, osc, ro = (p[i] for p in s._parts)
                nb0 = s.classes[i][1]
            else:
                idx = np.zeros(0, np.int64); sc = np.zeros(0, np.float32)
                mk = np.zeros(0, np.float32); osc = np.zeros((0, P), np.float32)
                ro = np.zeros((0, P), np.int64); nb0 = 0
            add = NB - nb0
            if add:
                idx = np.concatenate([idx, np.zeros(add * Dv * P, np.int64)])
                sc = np.concatenate([sc, np.zeros(add * Dv * P, np.float32)])
                mk = np.concatenate([mk, np.full(add * Dv * P, NEG, np.float32)])
                osc = np.concatenate([osc, np.ones((add, P), np.float32)])
                ro = np.concatenate([ro, np.zeros((add, P), np.int64)])
            # pad rows must be unique: rewrite non-real rows
            flat = ro.reshape(-1)
            realmask = np.zeros(len(flat), bool)
            nreal_here = s.classes[have[0]][2] if have else 0
            realmask[:nreal_here] = True
            flat[~realmask] = next_pad_row + np.arange((~realmask).sum())
            next_pad_row += (~realmask).sum()
            idx_l.append(idx); sc_l.append(sc); mk_l.append(mk)
            os_l.append(osc); ro_l.append(ro)
        s.idx = np.concatenate(idx_l) if idx_l else np.zeros(0, np.int64)
        s.scale = np.concatenate(sc_l) if sc_l else np.zeros(0, np.float32)
        s.mask = np.concatenate(mk_l) if mk_l else np.zeros(0, np.float32)
        s.outsc = np.concatenate(os_l, 0) if os_l else np.zeros((0, P), np.float32)
        s.rowoff = np.concatenate(ro_l, 0) if ro_l else np.zeros((0, P), np.int64)
        s.table_rows = ntiles * P
        # processed-order row of each dst row index
        pr = np.full(s._nreal, -1, np.int64)
        base = 0
        for ci, (Dv, NB) in enumerate(common):
            have = [i for i, c in enumerate(s.classes) if c[0] == Dv]
            if have:
                sel = s._sels[have[0]]
                pr[sel] = base + np.arange(len(sel))
            base += NB * P
        s.proc_row = pr
        if getattr(s, "contig", False):
            s.rowoff = np.arange(ntiles * P, dtype=np.int64).reshape(ntiles, P)
        assert s.idx.max(initial=0) < 32768
    return common, ntiles


def _emulate_layer(spec, common, src_tab, W, wsrep=None, wdrep=None, out=None):
    """Numpy mirror of the device layer; returns output table."""
    S = spec.idx
    G = src_tab[S]                      # [Stot,128]
    ntiles = sum(nb for _, nb in common)
    if out is None:
        out = np.zeros((max(ntiles * P, int(spec.rowoff.max()) + 1), HID), np.float32)
    pos = 0
    t = 0
    for Dv, NB in common:
        cnt = NB * Dv * P
        g = G[pos:pos + cnt].reshape(NB, Dv, P, HID)
        if spec.is_gat:
            mk = spec.mask[pos:pos + cnt].reshape(NB, Dv, P)
            a_s = (g * wsrep[0]).sum(-1)
            a_d = (g[:, 0] * wdrep[0]).sum(-1)
            e = a_s + a_d[:, None, :] + mk
            e = np.maximum(e, 0.2 * e)
            m = e.max(1, keepdims=True)
            w = np.exp(e - m)
            alpha = w / w.sum(1, keepdims=True)
            U = (alpha[..., None] * g).sum(1)     # [NB,P,HID]
        else:
            sc = spec.scale[pos:pos + cnt].reshape(NB, Dv, P)
            U = (sc[..., None] * g).sum(1)
        Y = (U @ W) * spec.outsc[t:t + NB][..., None]
        out[spec.rowoff[t:t + NB].reshape(-1)] = Y.reshape(-1, HID)
        pos += cnt
        t += NB
    return out


# ---------------------------------------------------------------- host prep

def _prep(inputs):
    B = int(inputs["num_graphs"])
    NC = 8
    Gper = B // NC
    lx = inputs["left_x"][:, 0].astype(np.int64)
    rx = inputs["right_x"][:, 0].astype(np.int64)
    N = lx.shape[0]
    onto = {}
    for nm, tbl, L in (("diag", "diag_table", 20000), ("proce", "proce_table", 8000),
                       ("atc", "atc_table", 4000)):
        onto[nm] = dict(N=inputs[tbl].shape[0], L=L,
                        e1=inputs[nm + "_e1"].astype(np.int64),
                        e2=inputs[nm + "_e2"].astype(np.int64),
                        map=inputs[nm + "_map"].astype(np.int64))
    # all_emb row layout
    offs = {"diag": 1, "proce": 1 + 20000, "atc": 1 + 28000}
    AE = 1 + 32000
    AEpad = -(-AE // NC) * NC
    SH = AEpad // NC
    # patient degrees
    sides = {}
    for sd, ek, xk, bk in (("L", "left_graph_index", "left_x", "left_x_batch"),
                           ("R", "right_graph_index", "right_x", "right_x_batch")):
        e = inputs[ek].astype(np.int64)
        batch = inputs[bk].astype(np.int64)
        cnt = np.bincount(batch, minlength=B)
        first = np.concatenate([[0], np.cumsum(cnt)[:-1]])
        deg = np.bincount(e[1], minlength=N) + 1
        dinv = (1.0 / np.sqrt(deg)).astype(np.float32)
        sides[sd] = dict(e=e, first=first, dinv=dinv,
                         x=lx if sd == "L" else rx)
    cores = [dict(layers={}) for _ in range(NC)]
    # ---- stage A per core
    for nm in ("diag", "proce", "atc"):
        o = onto[nm]
        No = o["N"]
        deg2 = np.bincount(o["e2"][1], minlength=No)
        specs1, specs2 = [], []
        stash = []
        for c in range(NC):
            lo, hi = c * SH, (c + 1) * SH
            rows = np.arange(max(lo, offs[nm]), min(hi, offs[nm] + o["L"]))
            dstn = o["map"][rows - offs[nm]]        # ontology nodes for slice rows
            out_rows = rows - lo                     # staging rows
            act = np.zeros(No, bool); act[dstn] = True
            m = act[o["e2"][1]]
            es, ed = o["e2"][0][m], o["e2"][1][m]
            noderow = np.full(No, -1, np.int64)
            noderow[dstn] = np.arange(len(dstn))
            ed_r = noderow[ed]
            A1 = np.unique(np.concatenate([es, dstn]))
            act1 = np.zeros(No, bool); act1[A1] = True
            m1 = act1[o["e1"][1]]
            es1, ed1 = o["e1"][0][m1], o["e1"][1][m1]
            ren1 = np.full(No, -1, np.int64)
            ren1[A1] = np.arange(len(A1))
            ident = np.arange(No)
            s1 = _build_layer(A1, np.arange(len(A1)), es1, ren1[ed1], ident, ident,
                              is_gat=True)
            s1.contig = True
            specs1.append(s1)
            stash.append((dstn, out_rows, es, ed_r, A1))
        c1, t1 = _uniformize(specs1)
        for c in range(NC):
            dstn, out_rows, es, ed_r, A1 = stash[c]
            proc1 = np.full(No, -1, np.int64)
            proc1[A1] = specs1[c].proc_row
            s2 = _build_layer(dstn, out_rows, es, ed_r, proc1, proc1, is_gat=True)
            specs2.append(s2)
        pbase = {"diag": 0, "proce": 1, "atc": 2}[nm]
        c2, t2 = _uniformize(specs2, pad_base=SH + pbase * 8192)
        for c in range(NC):
            cores[c]["layers"][nm + "1"] = (specs1[c], c1, t1)
            cores[c]["layers"][nm + "2"] = (specs2[c], c2, t2)
    # ---- patient per core
    for sd in ("L", "R"):
        S = sides[sd]
        e, dinv, x = S["e"], S["dinv"], S["x"]
        traces = []
        for c in range(NC):
            A3 = S["first"][c * Gper:(c + 1) * Gper]
            act = np.zeros(N, bool); act[A3] = True
            m3 = act[e[1]]
            es3, ed3 = e[0][m3], e[1][m3]
            A2 = np.unique(np.concatenate([es3, A3]))
            act2 = np.zeros(N, bool); act2[A2] = True
            m2 = act2[e[1]]
            es2, ed2 = e[0][m2], e[1][m2]
            A1 = np.unique(np.concatenate([es2, A2]))
            act1 = np.zeros(N, bool); act1[A1] = True
            m1 = act1[e[1]]
            es1, ed1 = e[0][m1], e[1][m1]
            traces.append((A3, es3, ed3, A2, es2, ed2, A1, es1, ed1))
        s1s, s2s, s3s = [], [], []
        for c in range(NC):
            A3, es3, ed3, A2, es2, ed2, A1, es1, ed1 = traces[c]
            r1 = np.full(N, -1, np.int64); r1[A1] = np.arange(len(A1))
            s1 = _build_layer(A1, np.arange(len(A1)), es1, r1[ed1],
                              x, x, slot_scale_src=dinv, slot_scale_self=dinv,
                              out_scale=dinv)
            s1.contig = True
            s1s.append(s1)
        cc1, tt1 = _uniformize(s1s)
        for c in range(NC):
            A3, es3, ed3, A2, es2, ed2, A1, es1, ed1 = traces[c]
            p1 = np.full(N, -1, np.int64); p1[A1] = s1s[c].proc_row
            r2 = np.full(N, -1, np.int64); r2[A2] = np.arange(len(A2))
            s2 = _build_layer(A2, np.arange(len(A2)), es2, r2[ed2],
                              p1, p1, slot_scale_src=dinv, slot_scale_self=dinv,
                              out_scale=dinv)
            s2.contig = True
            s2s.append(s2)
        cc2, tt2 = _uniformize(s2s)
        for c in range(NC):
            A3, es3, ed3, A2, es2, ed2, A1, es1, ed1 = traces[c]
            p2 = np.full(N, -1, np.int64); p2[A2] = s2s[c].proc_row
            s3 = _build_layer(A3, np.arange(Gper), es3,
                              np.searchsorted(A3, ed3), p2, p2,
                              slot_scale_src=dinv, slot_scale_self=dinv)
            s3s.append(s3)
        cc3, tt3 = _uniformize(s3s)
        for c, (sp, cc, tt) in ((c, z) for c in range(NC)
                                for z in [(s1s[c], cc1, tt1)]):
            pass
        for c in range(NC):
            cores[c]["layers"][sd + "1"] = (s1s[c], cc1, tt1)
            cores[c]["layers"][sd + "2"] = (s2s[c], cc2, tt2)
            cores[c]["layers"][sd + "3"] = (s3s[c], cc3, tt3)
    meta = dict(B=B, NC=NC, Gper=Gper, AEpad=AEpad, SH=SH, onto=onto, offs=offs)
    return cores, meta


# ------------------------------------------------------------ device builder

def _pack_idx(idx):
    S = len(idx)
    return np.tile(idx.astype(np.int16).reshape(S // 16, 16).T, (8, 1)).copy()


def _pack_f32(a):
    S = len(a)
    return a.astype(np.float32).reshape(S // P, P).T.copy()


def _build_and_run(cores, meta, inputs):
    global LAST_EXEC_TIME_NS
    import concourse.bacc as bacc
    import concourse.bass as bass
    import concourse.mybir as mybir
    import concourse.tile as tile
    from concourse.masks import make_identity
    from concourse.bass_utils import run_bass_kernel_spmd

    NCRS = meta["NC"]
    AEpad, SH = meta["AEpad"], meta["SH"]
    onto = meta["onto"]
    LORDER = ["diag1", "proce1", "atc1", "diag2", "proce2", "atc2",
              "L1", "L2", "L3", "R1", "R2", "R3"]
    SRC_OF = {"diag1": "diag_table", "proce1": "proce_table", "atc1": "atc_table",
              "diag2": "T_diag1", "proce2": "T_proce1", "atc2": "T_atc1",
              "L1": "AEFULL", "L2": "T_L1", "L3": "T_L2",
              "R1": "AEFULL", "R2": "T_R1", "R3": "T_R2"}
    W_OF = {"diag1": "diag_W", "proce1": "proce_W", "atc1": "atc_W",
            "diag2": "diag_W", "proce2": "proce_W", "atc2": "atc_W",
            "L1": "gcn_W1", "L2": "gcn_W2", "L3": "gcn_W3"}
    W_OF.update({"R1": "gcn_W1", "R2": "gcn_W2", "R3": "gcn_W3"})
    DST_OF = {"diag1": "T_diag1", "proce1": "T_proce1", "atc1": "T_atc1",
              "diag2": "STAGING", "proce2": "STAGING", "atc2": "STAGING",
              "L1": "T_L1", "L2": "T_L2", "L3": "T_lf",
              "R1": "T_R1", "R2": "T_R2", "R3": "T_rf"}

    c0 = cores[0]
    nc = bacc.Bacc("TRN2", target_bir_lowering=False, debug=False,
                   num_devices=NCRS, num_swdge_queues=NQ)
    dt = mybir.dt
    # inputs
    ext = {}
    for nm in ("diag", "proce", "atc"):
        ext[nm + "_table"] = nc.dram_tensor(nm + "_table", list(inputs[nm + "_table"].shape),
                                            dt.float32, kind="ExternalInput")
        for w in ("W", "ws", "wd"):
            ext[nm + "_" + w] = nc.dram_tensor(nm + "_" + w, [P, P], dt.float32,
                                               kind="ExternalInput")
    for i in (1, 2, 3):
        ext[f"gcn_W{i}"] = nc.dram_tensor(f"gcn_W{i}", [P, P], dt.float32,
                                          kind="ExternalInput")
    ext["spec_patch"] = nc.dram_tensor("spec_patch", [1, HID], dt.float32,
                                       kind="ExternalInput")
    per_layer_ext = {}
    for ln in LORDER:
        spec, common, ntl = c0["layers"][ln]
        Stot = len(spec.idx)
        d = {}
        d["idx"] = nc.dram_tensor(ln + "_idx", [P, Stot // 16], dt.int16, kind="ExternalInput")
        d["sca"] = nc.dram_tensor(ln + "_sca", [P, Stot // P], dt.float32, kind="ExternalInput")
        d["osc"] = nc.dram_tensor(ln + "_osc", [P, ntl], dt.float32, kind="ExternalInput")
        d["row"] = nc.dram_tensor(ln + "_row", [P, ntl], dt.int32, kind="ExternalInput")
        per_layer_ext[ln] = d
    # internal tables
    tabs = {}
    for ln in LORDER:
        _, _, ntl = c0["layers"][ln]
        name = DST_OF[ln]
        rows = ntl * P
        if name == "STAGING":
            rows = max(rows, SH + 3 * 8192 + 8192)
        if name not in tabs or tabs[name][1] < rows:
            tabs[name] = (name, rows)
    tabs["STAGING"] = ("STAGING", max(tabs["STAGING"][1], SH + 4 * 8192))
    dev_tabs = {}
    for name, rows in tabs.values():
        dev_tabs[name] = nc.dram_tensor(name, [rows, HID], dt.float32)
    dev_tabs["AEFULL"] = nc.dram_tensor("AEFULL", [AEpad, HID], dt.float32,
                                        addr_space="Shared")
    out_t = nc.dram_tensor("cos_out", [P, meta["Gper"] // P], dt.float32,
                           kind="ExternalOutput")

    qrr = [0]

    with tile.TileContext(nc) as tc:
        import contextlib
        with contextlib.ExitStack() as ctx:
            cpool = ctx.enter_context(tc.tile_pool(name="const", bufs=1))
            gpool = ctx.enter_context(tc.tile_pool(name="g", bufs=10))
            gbig = ctx.enter_context(tc.tile_pool(name="gb", bufs=2))
            spool = ctx.enter_context(tc.tile_pool(name="s", bufs=4))
            mpool = ctx.enter_context(tc.tile_pool(name="m", bufs=5))
            pps = ctx.enter_context(tc.tile_pool(name="ps", bufs=3, space="PSUM"))
            pps2 = ctx.enter_context(tc.tile_pool(name="ps2", bufs=2, space="PSUM"))
            ident = cpool.tile([P, P], dt.float32)
            make_identity(nc, ident[:])
            wsb = {}
            for key in set(W_OF.values()) | {nm + "_" + w for nm in ("diag", "proce", "atc") for w in ("ws", "wd")}:
                t = cpool.tile([P, P], dt.float32, tag="w_" + key)
                nc.sync.dma_start(t[:], ext[key][:])
                wsb[key] = t
            # spec patch first (core0 staging row0; others overwritten by scatter)
            pt = cpool.tile([1, HID], dt.float32)
            nc.sync.dma_start(pt[:], ext["spec_patch"][:])
            nc.sync.dma_start(dev_tabs["STAGING"][0:1, :], pt[:])

            def emit_layer(ln):
                spec, common, ntl = c0["layers"][ln]
                is_gat = spec.is_gat
                exts = per_layer_ext[ln]
                Stot = len(spec.idx)
                it = mpool.tile([P, Stot // 16], dt.int16, tag="idx")
                nc.sync.dma_start(it[:], exts["idx"][:])
                sct = mpool.tile([P, Stot // P], dt.float32, tag="sca")
                nc.sync.dma_start(sct[:], exts["sca"][:])
                osct = mpool.tile([P, ntl], dt.float32, tag="osc")
                nc.sync.dma_start(osct[:], exts["osc"][:])
                rowt = mpool.tile([P, ntl], dt.int32, tag="row")
                nc.sync.dma_start(rowt[:], exts["row"][:])
                src_tab = dev_tabs.get(SRC_OF[ln]) or ext[SRC_OF[ln]]
                dst_tab = dev_tabs[DST_OF[ln]]
                Wt = wsb[W_OF[ln]]
                if is_gat:
                    base = ln[:-1]
                    ws_t, wd_t = wsb[base + "_ws"], wsb[base + "_wd"]
                pos = 0
                tix = 0
                for Dv, NB in common:
                    gb = max(1, 8 // Dv)
                    nch = max(1, (Dv * gb) // 8)
                    for g0 in range(NB // gb):
                        Ssl = gb * Dv            # slots per group
                        if Ssl <= 8:
                            gbuf = gpool.tile([P, Ssl, HID], dt.float32, tag="gbuf")
                        else:
                            gbuf = gbig.tile([P, Ssl, HID], dt.float32, tag="gbufbig")
                        for k in range(nch):
                            cbase = (pos + g0 * Ssl * P + k * CHUNK) // 16
                            nc.gpsimd.dma_gather(
                                out_ap=gbuf[:, k * 8:(k + 1) * 8, :],
                                in_ap=src_tab[:], idxs_ap=it[:, cbase:cbase + 64],
                                num_idxs=CHUNK, num_idxs_reg=CHUNK, elem_size=HID,
                                queue_num=qrr[0] % NQ)
                            qrr[0] += 1
                        gv = gbuf[:].rearrange("p (g d) f -> p g d f", d=Dv)
                        if is_gat:
                            tmp = spool.tile([P, Ssl, HID], dt.float32, tag="att_tmp")
                            nc.vector.tensor_tensor(
                                out=tmp[:], in0=gbuf[:],
                                in1=ws_t[:].unsqueeze(1).to_broadcast([P, Ssl, HID]),
                                op=mybir.AluOpType.mult)
                            a_s = spool.tile([P, gb, Dv], dt.float32, tag="a_s")
                            nc.vector.tensor_reduce(
                                out=a_s[:].rearrange("p g d -> p (g d)"), in_=tmp[:],
                                axis=mybir.AxisListType.X, op=mybir.AluOpType.add)
                            tmpd = spool.tile([P, gb, HID], dt.float32, tag="tmpd")
                            nc.vector.tensor_tensor(
                                out=tmpd[:], in0=gv[:, :, 0, :],
                                in1=wd_t[:].unsqueeze(1).to_broadcast([P, gb, HID]),
                                op=mybir.AluOpType.mult)
                            a_d = spool.tile([P, gb, 1], dt.float32, tag="a_d")
                            nc.vector.tensor_reduce(
                                out=a_d[:, :, 0], in_=tmpd[:],
                                axis=mybir.AxisListType.X, op=mybir.AluOpType.add)
                            e = spool.tile([P, gb, Dv], dt.float32, tag="e")
                            nc.vector.tensor_tensor(
                                out=e[:], in0=a_s[:],
                                in1=a_d[:].to_broadcast([P, gb, Dv]),
                                op=mybir.AluOpType.add)
                            mrow = (pos + g0 * Ssl * P) // P
                            nc.vector.tensor_tensor(
                                out=e[:], in0=e[:],
                                in1=sct[:, mrow:mrow + Ssl].rearrange("p (g d) -> p g d", d=Dv),
                                op=mybir.AluOpType.add)
                            nc.vector.scalar_tensor_tensor(
                                out=e[:], in0=e[:], scalar=0.2, in1=e[:],
                                op0=mybir.AluOpType.mult, op1=mybir.AluOpType.max)
                            mx = spool.tile([P, gb, 1], dt.float32, tag="mx")
                            nc.vector.tensor_reduce(out=mx[:, :, 0], in_=e[:],
                                axis=mybir.AxisListType.X, op=mybir.AluOpType.max)
                            nc.vector.tensor_tensor(
                                out=e[:], in0=e[:], in1=mx[:].to_broadcast([P, gb, Dv]),
                                op=mybir.AluOpType.subtract)
                            nc.scalar.activation(out=e[:], in_=e[:],
                                func=mybir.ActivationFunctionType.Exp)
                            den = spool.tile([P, gb, 1], dt.float32, tag="den")
                            nc.vector.tensor_reduce(out=den[:, :, 0], in_=e[:],
                                axis=mybir.AxisListType.X, op=mybir.AluOpType.add)
                            nc.vector.reciprocal(out=den[:], in_=den[:])
                            nc.vector.tensor_tensor(
                                out=e[:], in0=e[:], in1=den[:].to_broadcast([P, gb, Dv]),
                                op=mybir.AluOpType.mult)
                            alpha_b = e[:].rearrange("p g d -> p (g d)").unsqueeze(2) \
                                          .to_broadcast([P, Ssl, HID])
                        else:
                            srow = (pos + g0 * Ssl * P) // P
                            alpha_b = sct[:, srow:srow + Ssl].unsqueeze(2) \
                                          .to_broadcast([P, Ssl, HID])
                        nc.vector.tensor_tensor(out=gbuf[:], in0=gbuf[:], in1=alpha_b,
                                                op=mybir.AluOpType.mult)
                        # segsum tree over d
                        step = Dv
                        while step > 1:
                            h = step // 2
                            nc.vector.tensor_tensor(
                                out=gv[:, :, 0:h, :], in0=gv[:, :, 0:h, :],
                                in1=gv[:, :, h:step, :], op=mybir.AluOpType.add)
                            step = h
                        for b in range(gb):
                            ublk = gv[:, b, 0, :]
                            psA = pps.tile([P, P], dt.float32, tag="psA", space="PSUM")
                            nc.tensor.transpose(out=psA[:], in_=ublk, identity=ident[:])
                            uf = spool.tile([P, P], dt.float32, tag="uf")
                            nc.vector.tensor_copy(uf[:], psA[:])
                            psB = pps.tile([P, P], dt.float32, tag="psB", space="PSUM")
                            nc.tensor.matmul(out=psB[:], lhsT=Wt[:], rhs=uf[:],
                                             start=True, stop=True)
                            hf = spool.tile([P, P], dt.float32, tag="hf")
                            nc.vector.tensor_copy(hf[:], psB[:])
                            psC = pps2.tile([P, P], dt.float32, tag="psC", space="PSUM")
                            nc.tensor.transpose(out=psC[:], in_=hf[:], identity=ident[:])
                            on = spool.tile([P, P], dt.float32, tag="on")
                            nc.vector.tensor_scalar(
                                out=on[:], in0=psC[:], scalar1=osct[:, tix:tix + 1],
                                scalar2=None, op0=mybir.AluOpType.mult)
                            if spec.contig:
                                nc.sync.dma_start(
                                    dst_tab[tix * P:(tix + 1) * P, :], on[:])
                            else:
                                nc.gpsimd.indirect_dma_start(
                                    out=dst_tab[:], out_offset=bass.IndirectOffsetOnAxis(
                                        ap=rowt[:, tix:tix + 1], axis=0),
                                    in_=on[:], in_offset=None)
                            tix += 1
                    pos += NB * Dv * P

            for ln in ("diag1", "proce1", "diag2", "atc1", "proce2", "atc2"):
                emit_layer(ln)
            nc.gpsimd.collective_compute(
                "AllGather", mybir.AluOpType.bypass,
                replica_groups=[list(range(NCRS))],
                ins=[dev_tabs["STAGING"][0:SH, :]], outs=[dev_tabs["AEFULL"][:]])
            for ln in ("L1", "R1", "L2", "R2", "L3", "R3"):
                emit_layer(ln)
            # cosine
            GT = meta["Gper"] // P
            lf = spool.tile([P, GT, HID], dt.float32, tag="lf")
            rf = spool.tile([P, GT, HID], dt.float32, tag="rf")
            nc.sync.dma_start(lf[:], dev_tabs["T_lf"][0:meta["Gper"], :]
                              .rearrange("(n p) f -> p n f", p=P))
            nc.sync.dma_start(rf[:], dev_tabs["T_rf"][0:meta["Gper"], :]
                              .rearrange("(n p) f -> p n f", p=P))
            pr = spool.tile([P, GT, HID], dt.float32, tag="pr")
            num = spool.tile([P, GT], dt.float32, tag="num")
            nc.vector.tensor_tensor(out=pr[:], in0=lf[:], in1=rf[:], op=mybir.AluOpType.mult)
            nc.vector.tensor_reduce(out=num[:], in_=pr[:], axis=mybir.AxisListType.X,
                                    op=mybir.AluOpType.add)
            nl = spool.tile([P, GT], dt.float32, tag="nl")
            nc.vector.tensor_tensor(out=pr[:], in0=lf[:], in1=lf[:], op=mybir.AluOpType.mult)
            nc.vector.tensor_reduce(out=nl[:], in_=pr[:], axis=mybir.AxisListType.X,
                                    op=mybir.AluOpType.add)
            nr = spool.tile([P, GT], dt.float32, tag="nr")
            nc.vector.tensor_tensor(out=pr[:], in0=rf[:], in1=rf[:], op=mybir.AluOpType.mult)
            nc.vector.tensor_reduce(out=nr[:], in_=pr[:], axis=mybir.AxisListType.X,
                                    op=mybir.AluOpType.add)
            nc.vector.tensor_tensor(out=nl[:], in0=nl[:], in1=nr[:], op=mybir.AluOpType.mult)
            nc.scalar.activation(out=nl[:], in_=nl[:], func=mybir.ActivationFunctionType.Sqrt)
            nc.vector.reciprocal(out=nl[:], in_=nl[:])
            nc.vector.tensor_tensor(out=num[:], in0=num[:], in1=nl[:], op=mybir.AluOpType.mult)
            nc.sync.dma_start(out_t[:], num[:])
    nc.compile()

    # per-core input maps
    in_maps = []
    for c in range(NCRS):
        im = {}
        for nm in ("diag", "proce", "atc"):
            im[nm + "_table"] = np.ascontiguousarray(inputs[nm + "_table"], np.float32)
            W = np.asarray(inputs[nm + "_W"], np.float32)
            ws = (W @ np.asarray(inputs[nm + "_asrc"], np.float32))
            wd = (W @ np.asarray(inputs[nm + "_adst"], np.float32))
            im[nm + "_W"] = W
            im[nm + "_ws"] = np.tile(ws[None, :], (P, 1)).astype(np.float32)
            im[nm + "_wd"] = np.tile(wd[None, :], (P, 1)).astype(np.float32)
        for i in (1, 2, 3):
            im[f"gcn_W{i}"] = np.asarray(inputs[f"gcn_W{i}"], np.float32)
        im["spec_patch"] = np.asarray(inputs["spec_emb"], np.float32).reshape(1, HID)
        for ln in LORDER:
            spec, common, ntl = cores[c]["layers"][ln]
            im[ln + "_idx"] = _pack_idx(spec.idx)
            im[ln + "_sca"] = _pack_f32(spec.mask if spec.is_gat else spec.scale)
            im[ln + "_osc"] = spec.outsc.T.copy().astype(np.float32)
            im[ln + "_row"] = spec.rowoff.T.copy().astype(np.int32)
        in_maps.append(im)
    res = run_bass_kernel_spmd(nc, in_maps, core_ids=list(range(NCRS)),
                               trace=bool(os.environ.get("KBENCH_TRACE")))
    LAST_EXEC_TIME_NS = res.exec_time_ns
    out = np.empty(meta["B"], np.float32)
    Gper = meta["Gper"]
    for c in range(NCRS):
        o = res.results[c]["cos_out"]          # [128, Gper//128]; graph r at [r%128, r//128]
        out[c * Gper:(c + 1) * Gper] = o.T.reshape(-1)
    return out


def emulate(inputs):
    """Pure-numpy mirror of the device program (for host-array validation)."""
    cores, meta = _prep(inputs)
    B, NCn, Gper = meta["B"], meta["NC"], meta["Gper"]
    AEpad, SH = meta["AEpad"], meta["SH"]
    wrep = {}
    for nm in ("diag", "proce", "atc"):
        W = np.asarray(inputs[nm + "_W"], np.float32)
        wrep[nm] = (W, np.tile((W @ inputs[nm + "_asrc"])[None], (P, 1)),
                    np.tile((W @ inputs[nm + "_adst"])[None], (P, 1)))
    ae_full = np.zeros((AEpad, HID), np.float32)
    lf_all, rf_all = {}, {}
    for c in range(NCn):
        L = cores[c]["layers"]
        stag = np.zeros((SH + 4 * 8192, HID), np.float32)
        stag[0] = np.asarray(inputs["spec_emb"], np.float32)
        for nm in ("diag", "proce", "atc"):
            W, ws, wd = wrep[nm]
            t1 = _emulate_layer(L[nm + "1"][0], L[nm + "1"][1],
                                np.asarray(inputs[nm + "_table"], np.float32), W, ws, wd)
            _emulate_layer(L[nm + "2"][0], L[nm + "2"][1], t1, W, ws, wd, out=stag)
        ae_full[c * SH:(c + 1) * SH] = stag[:SH]
    for c in range(NCn):
        L = cores[c]["layers"]
        for sd, store in (("L", lf_all), ("R", rf_all)):
            t1 = _emulate_layer(L[sd + "1"][0], L[sd + "1"][1], ae_full,
                                np.asarray(inputs["gcn_W1"], np.float32))
            t2 = _emulate_layer(L[sd + "2"][0], L[sd + "2"][1], t1,
                                np.asarray(inputs["gcn_W2"], np.float32))
            t3 = _emulate_layer(L[sd + "3"][0], L[sd + "3"][1], t2,
                                np.asarray(inputs["gcn_W3"], np.float32))
            store[c] = t3[:Gper]
    out = np.empty(B, np.float32)
    for c in range(NCn):
        lf, rf = lf_all[c], rf_all[c]
        num = (lf * rf).sum(-1)
        den = np.sqrt((lf * lf).sum(-1) * (rf * rf).sum(-1))
        out[c * Gper:(c + 1) * Gper] = num / den
    return out


def kernel(**inputs):
    for k in ("diag_b", "proce_b", "atc_b", "gcn_b1", "gcn_b2", "gcn_b3"):
        assert np.abs(np.asarray(inputs[k])).max() == 0.0, f"nonzero bias {k}"
    cores, meta = _prep(inputs)
    out = _build_and_run(cores, meta, inputs)
    return out.astype(np.float32)
